# revision 12
# baseline (speedup 1.0000x reference)
"""Distributed single-head attention for TRN2 (8 NeuronCores).

Reference computation (per batch b):
    q = x @ Wq; k = x @ Wk; v = x @ Wv          (x: [S, E])
    s = (q @ k.T) / sqrt(DK) - 1e15 * mask
    out = softmax(s, axis=-1) @ v               ([S, DV])

Sharding: 8 cores = 4 batches x 2 sequence halves. Each core computes
attention for 1024 queries of one batch; K/V are recomputed per core from
the full sequence (cheap vs. the attention matmuls, avoids collectives).

Per-core layout choices (host prepares these in kernel()):
  - xt  [E, S]  bf16: x_b^T with the sequence permuted so this core's
                query half occupies columns [0, 1024). K/V are computed
                over the permuted order, which is harmless because
                softmax-attention is permutation invariant over keys.
  - wq  [E, DK] bf16: Wq pre-scaled by 1/sqrt(DK).
  - mt  [S, SQ] bf16: mask[b, q0:q0+SQ, :] transposed to [key, query]
                with keys permuted identically to xt's columns.
  - out [DV, SQ] f32: O^T; host transposes back.

On-core dataflow (all matmul contractions on the 128-partition dim):
  QT[d,q], KT[d,k], VT[d,k] projections -> PE transpose VT -> V[k,d]
  per key-tile t: ST[k128,q] = KT_t^T QT (PE) -> masked = mt*-1e15 + ST
  (DVE) -> P = exp(masked) bf16 (ACT) -> rowsum over k (GPSIMD C-reduce)
  and OT[dv,q] += V_t^T P (PE, PSUM accumulate). Normalization happens
  once at the end: OT * broadcast(1/rowsum).
"""

import math
from contextlib import ExitStack

import ml_dtypes
import numpy as np

import concourse.bass as bass
import concourse.tile as tile
from concourse import bacc, masks, mybir
from concourse.bass_utils import run_bass_kernel_spmd

B, S, E, DK, DV = 4, 2048, 1024, 128, 128
SQ = S // 2  # queries per core
P = 128  # SBUF partitions
EC = E // P  # contraction chunks for projections
KTILES = S // P  # key tiles
NEG = -1.0e15

f32 = mybir.dt.float32
bf16 = mybir.dt.bfloat16

# test.py pokes these to get profiling info
TRACE = False
LAST_RESULT = None


def build():
    nc = bacc.Bacc()
    xt = nc.declare_dram_parameter("xt", [E, S], bf16, isOutput=False)
    wq = nc.declare_dram_parameter("wq", [E, DK], bf16, isOutput=False)
    wk = nc.declare_dram_parameter("wk", [E, DK], bf16, isOutput=False)
    wv = nc.declare_dram_parameter("wv", [E, DV], bf16, isOutput=False)
    mt = nc.declare_dram_parameter("mt", [S, SQ], bf16, isOutput=False)
    out = nc.declare_dram_parameter("out", [DV, SQ], f32, isOutput=True)

    with ExitStack() as ctx:
        tc = ctx.enter_context(tile.TileContext(nc))
        const_pool = ctx.enter_context(tc.tile_pool(name="const", bufs=1))
        in_pool = ctx.enter_context(tc.tile_pool(name="inputs", bufs=1))
        proj_sb = ctx.enter_context(tc.tile_pool(name="proj", bufs=1))
        st_ps = ctx.enter_context(tc.tile_pool(name="st_ps", bufs=2, space="PSUM"))
        ot_ps = ctx.enter_context(tc.tile_pool(name="ot_ps", bufs=1, space="PSUM"))
        msk_pool = ctx.enter_context(tc.tile_pool(name="masked", bufs=3))
        p_pool = ctx.enter_context(tc.tile_pool(name="p", bufs=3))
        stat = ctx.enter_context(tc.tile_pool(name="stat", bufs=1))
        proj_ctx = ctx.enter_context(ExitStack())
        proj_ps = proj_ctx.enter_context(
            tc.tile_pool(name="proj_ps", bufs=2, space="PSUM")
        )

        ident = const_pool.tile([P, P], bf16)
        masks.make_identity(nc, ident[:])
        ones_col = const_pool.tile([P, 1], bf16)
        nc.gpsimd.memset(ones_col[:], 1.0)
        ones_row = const_pool.tile([1, P], f32)
        nc.gpsimd.memset(ones_row[:], 1.0)

        # --- load inputs ---
        x_sb = in_pool.tile([P, EC * S], bf16)
        for c in range(EC):
            nc.sync.dma_start(x_sb[:, c * S : (c + 1) * S], xt[c * P : (c + 1) * P, :])
        m_sb = in_pool.tile([P, KTILES * SQ], bf16)
        for t in range(KTILES):
            nc.sync.dma_start(
                m_sb[:, t * SQ : (t + 1) * SQ], mt[t * P : (t + 1) * P, :]
            )
        w_sb = {}
        for name, w in (("wq", wq), ("wk", wk), ("wv", wv)):
            wt = in_pool.tile([P, EC * DK], bf16, tag=name)
            w_sb[name] = wt
            for c in range(EC):
                nc.sync.dma_start(
                    w_sb[name][:, c * DK : (c + 1) * DK], w[c * P : (c + 1) * P, :]
                )

        # --- projections: QT [d, q], KT [d, k], VT [d, k] (bf16 in SBUF) ---
        qt_sb = proj_sb.tile([P, SQ], bf16)
        kt_sb = proj_sb.tile([P, S], bf16)
        vt_sb = proj_sb.tile([P, S], bf16)

        def project(dst, w_tile, ncols):
            for j in range(ncols // 512):
                ps = proj_ps.tile([P, 512], f32, tag="pps")
                for c in range(EC):
                    nc.tensor.matmul(
                        ps[:],
                        w_tile[:, c * DK : (c + 1) * DK],
                        x_sb[:, c * S + j * 512 : c * S + (j + 1) * 512],
                        start=(c == 0),
                        stop=(c == EC - 1),
                    )
                nc.any.tensor_copy(dst[:, j * 512 : (j + 1) * 512], ps[:])

        project(qt_sb, w_sb["wq"], SQ)
        project(kt_sb, w_sb["wk"], S)
        project(vt_sb, w_sb["wv"], S)

        # --- V natural layout [k, dv]: PE-transpose VT tile by tile ---
        v_sb = proj_sb.tile([P, S], bf16)  # tile t at columns [t*DV, (t+1)*DV)
        for t in range(KTILES):
            vp = proj_ps.tile([P, P], bf16, tag="pps")
            nc.tensor.transpose(vp[:], vt_sb[:, t * P : (t + 1) * P], ident[:])
            nc.any.tensor_copy(v_sb[:, t * DV : (t + 1) * DV], vp[:])
        proj_ctx.close()  # free projection PSUM banks for the attention loop
        rs_pool = ctx.enter_context(tc.tile_pool(name="rs_ps", bufs=1, space="PSUM"))

        # --- attention over key tiles ---
        ot = ot_ps.tile([P, SQ], f32)  # OT [dv, q] accumulator
        rs = rs_pool.tile([1, SQ], f32)  # rowsum of exp(scores) per query
        for t in range(KTILES):
            st = st_ps.tile([P, SQ], f32, tag="st")  # [k128, q]
            for j in range(2):
                nc.tensor.matmul(
                    st[:, j * 512 : (j + 1) * 512],
                    kt_sb[:, t * P : (t + 1) * P],
                    qt_sb[:, j * 512 : (j + 1) * 512],
                    start=True,
                    stop=True,
                )
            msk = msk_pool.tile([P, SQ], f32, tag="masked")
            nc.vector.scalar_tensor_tensor(
                msk[:],
                m_sb[:, t * SQ : (t + 1) * SQ],
                NEG,
                st[:],
                op0=mybir.AluOpType.mult,
                op1=mybir.AluOpType.add,
            )
            p = p_pool.tile([P, SQ], bf16, tag="p")
            nc.scalar.activation(p[:], msk[:], mybir.ActivationFunctionType.Exp)
            for j in range(2):
                nc.tensor.matmul(
                    rs[:, j * 512 : (j + 1) * 512],
                    ones_col[:],
                    p[:, j * 512 : (j + 1) * 512],
                    start=(t == 0),
                    stop=(t == KTILES - 1),
                )
                nc.tensor.matmul(
                    ot[:, j * 512 : (j + 1) * 512],
                    v_sb[:, t * DV : (t + 1) * DV],
                    p[:, j * 512 : (j + 1) * 512],
                    start=(t == 0),
                    stop=(t == KTILES - 1),
                )

        # --- normalize and store ---
        rcp = stat.tile([1, SQ], f32)
        nc.vector.reciprocal(rcp[:], rs[:])
        rep = st_ps.tile([P, SQ], f32, tag="st")  # 1/rowsum replicated to 128 rows
        for j in range(2):
            nc.tensor.matmul(
                rep[:, j * 512 : (j + 1) * 512],
                ones_row[:],
                rcp[:, j * 512 : (j + 1) * 512],
                start=True,
                stop=True,
            )
        rep_sb = stat.tile([P, SQ], f32)
        nc.scalar.copy(rep_sb[:], rep[:])
        ot_n = stat.tile([P, SQ], f32)
        nc.vector.tensor_tensor(ot_n[:], ot[:], rep_sb[:], op=mybir.AluOpType.mult)
        nc.sync.dma_start(out[:, :], ot_n[:])

    nc.compile()
    return nc


_NC_CACHE = None


def kernel(inputs, mask, Wq, Wk, Wv):
    global _NC_CACHE, LAST_RESULT
    inputs = np.asarray(inputs)
    mask = np.asarray(mask)
    bf = ml_dtypes.bfloat16
    scale = np.float32(1.0 / math.sqrt(DK))
    wq_h = (np.asarray(Wq) * scale).astype(bf)
    wk_h = np.asarray(Wk).astype(bf)
    wv_h = np.asarray(Wv).astype(bf)

    if _NC_CACHE is None:
        _NC_CACHE = build()
    nc = _NC_CACHE

    in_maps = []
    for core in range(8):
        b, h = divmod(core, 2)
        q0 = h * SQ
        idx = np.r_[q0:S, 0:q0]  # rotate so this core's queries come first
        xb = inputs[b]  # [S, E] f32
        xt_core = np.ascontiguousarray(xb[idx].T).astype(bf)  # [E, S]
        mt_core = np.ascontiguousarray(
            mask[b, q0 : q0 + SQ, :][:, idx].T
        ).astype(bf)  # [S, SQ]
        in_maps.append(
            {"xt": xt_core, "wq": wq_h, "wk": wk_h, "wv": wv_h, "mt": mt_core}
        )

    res = run_bass_kernel_spmd(nc, in_maps, list(range(8)), trace=TRACE)
    LAST_RESULT = res
    outp = np.empty((B, S, DV), np.float32)
    for core in range(8):
        b, h = divmod(core, 2)
        q0 = h * SQ
        outp[b, q0 : q0 + SQ, :] = np.asarray(res.results[core]["out"]).T
    return outp


# revision 13
# speedup vs baseline: 1.0160x; 1.0160x over previous
"""Distributed single-head attention for TRN2 (8 NeuronCores).

Reference computation (per batch b):
    q = x @ Wq; k = x @ Wk; v = x @ Wv          (x: [S, E])
    s = (q @ k.T) / sqrt(DK) - 1e15 * mask
    out = softmax(s, axis=-1) @ v               ([S, DV])

Sharding: 8 cores = 4 batches x 2 sequence halves. Each core computes
attention for 1024 queries of one batch; K/V are recomputed per core from
the full sequence (cheap vs. the attention matmuls, avoids collectives).

Per-core layout choices (host prepares these in kernel()):
  - xt  [E, S]  bf16: x_b^T with the sequence permuted so this core's
                query half occupies columns [0, 1024). K/V are computed
                over the permuted order, which is harmless because
                softmax-attention is permutation invariant over keys.
  - wq  [E, DK] bf16: Wq pre-scaled by 1/sqrt(DK).
  - mt  [S, SQ] bf16: mask[b, q0:q0+SQ, :] transposed to [key, query]
                with keys permuted identically to xt's columns.
  - out [DV, SQ] f32: O^T; host transposes back.

On-core dataflow (all matmul contractions on the 128-partition dim):
  QT[d,q], KT[d,k], VT[d,k] projections -> PE transpose VT -> V[k,d]
  per key-tile t: ST[k128,q] = KT_t^T QT (PE) -> masked = mt*-1e15 + ST
  (DVE) -> P = exp(masked) bf16 (ACT) -> rowsum over k (GPSIMD C-reduce)
  and OT[dv,q] += V_t^T P (PE, PSUM accumulate). Normalization happens
  once at the end: OT * broadcast(1/rowsum).
"""

import math
from contextlib import ExitStack

import ml_dtypes
import numpy as np

import concourse.bass as bass
import concourse.tile as tile
from concourse import bacc, masks, mybir
from concourse.bass_utils import run_bass_kernel_spmd

B, S, E, DK, DV = 4, 2048, 1024, 128, 128
SQ = S // 2  # queries per core
P = 128  # SBUF partitions
EC = E // P  # contraction chunks for projections
KTILES = S // P  # key tiles
NEG = -1.0e15

f32 = mybir.dt.float32
bf16 = mybir.dt.bfloat16

# test.py pokes these to get profiling info
TRACE = False
LAST_RESULT = None


def build():
    nc = bacc.Bacc()
    xt = nc.declare_dram_parameter("xt", [E, S], bf16, isOutput=False)
    wq = nc.declare_dram_parameter("wq", [E, DK], bf16, isOutput=False)
    wk = nc.declare_dram_parameter("wk", [E, DK], bf16, isOutput=False)
    wv = nc.declare_dram_parameter("wv", [E, DV], bf16, isOutput=False)
    mt = nc.declare_dram_parameter("mt", [S, SQ], bf16, isOutput=False)
    out = nc.declare_dram_parameter("out", [DV, SQ], f32, isOutput=True)

    with ExitStack() as ctx:
        tc = ctx.enter_context(tile.TileContext(nc))
        const_pool = ctx.enter_context(tc.tile_pool(name="const", bufs=1))
        in_pool = ctx.enter_context(tc.tile_pool(name="inputs", bufs=1))
        proj_sb = ctx.enter_context(tc.tile_pool(name="proj", bufs=1))
        msk_pool = ctx.enter_context(tc.tile_pool(name="masked", bufs=3))
        p_pool = ctx.enter_context(tc.tile_pool(name="p", bufs=3))
        stat = ctx.enter_context(tc.tile_pool(name="stat", bufs=1))
        proj_ctx = ctx.enter_context(ExitStack())
        proj_ps = proj_ctx.enter_context(
            tc.tile_pool(name="proj_ps", bufs=4, space="PSUM")
        )

        ones_col = const_pool.tile([P, 1], bf16)
        nc.gpsimd.memset(ones_col[:], 1.0)
        ones_row = const_pool.tile([1, P], f32)
        nc.gpsimd.memset(ones_row[:], 1.0)

        # --- load weights first (tiny; PE's first matmuls need them) ---
        w_sb = {}
        for name, w in (("wq", wq), ("wk", wk), ("wv", wv)):
            wt = in_pool.tile([P, EC * DK], bf16, tag=name)
            w_sb[name] = wt
            for c in range(EC):
                nc.sync.dma_start(
                    w_sb[name][:, c * DK : (c + 1) * DK], w[c * P : (c + 1) * P, :]
                )
        x_sb = in_pool.tile([P, EC * S], bf16)
        for c in range(EC):
            nc.sync.dma_start(x_sb[:, c * S : (c + 1) * S], xt[c * P : (c + 1) * P, :])

        # --- projections: QT [d, q], KT [d, k], VT [d, k] (bf16 in SBUF) ---
        # Contraction chunk c is the outer loop so the first matmul only
        # needs x chunk 0, not all eight.
        qt_sb = proj_sb.tile([P, SQ], bf16)
        kt_sb = proj_sb.tile([P, S], bf16)
        vt_sb = proj_sb.tile([P, S], bf16)

        def project(dst, w_tile, ncols):
            nj = ncols // 512
            pss = []
            for j in range(nj):
                ps = proj_ps.tile([P, 512], f32, tag="pps")
                pss.append(ps)
            for c in range(EC):
                for j in range(nj):
                    nc.tensor.matmul(
                        pss[j][:],
                        w_tile[:, c * DK : (c + 1) * DK],
                        x_sb[:, c * S + j * 512 : c * S + (j + 1) * 512],
                        start=(c == 0),
                        stop=(c == EC - 1),
                    )
            for j in range(nj):
                nc.any.tensor_copy(dst[:, j * 512 : (j + 1) * 512], pss[j][:])

        project(qt_sb, w_sb["wq"], SQ)
        project(kt_sb, w_sb["wk"], S)
        project(vt_sb, w_sb["wv"], S)
        proj_ctx.close()  # free projection PSUM banks for the attention loop

        # --- V natural layout [k, dv] via DMA transpose (no PE time) ---
        v_sb = proj_sb.tile([P, S], bf16)  # tile t at columns [t*DV, (t+1)*DV)
        for t in range(KTILES):
            nc.sync.dma_start_transpose(
                v_sb[:, t * DV : (t + 1) * DV], vt_sb[:, t * P : (t + 1) * P]
            )

        # --- mask loads (needed only by the attention loop) ---
        m_sb = in_pool.tile([P, KTILES * SQ], bf16)
        for t in range(KTILES):
            nc.sync.dma_start(
                m_sb[:, t * SQ : (t + 1) * SQ], mt[t * P : (t + 1) * P, :]
            )

        st_ps = ctx.enter_context(tc.tile_pool(name="st_ps", bufs=2, space="PSUM"))
        ot_ps = ctx.enter_context(tc.tile_pool(name="ot_ps", bufs=1, space="PSUM"))
        rs_pool = ctx.enter_context(tc.tile_pool(name="rs_ps", bufs=1, space="PSUM"))

        # --- attention over key tiles ---
        ot = ot_ps.tile([P, SQ], f32)  # OT [dv, q] accumulator
        rs = rs_pool.tile([1, SQ], f32)  # rowsum of exp(scores) per query
        for t in range(KTILES):
            st = st_ps.tile([P, SQ], f32, tag="st")  # [k128, q]
            for j in range(2):
                nc.tensor.matmul(
                    st[:, j * 512 : (j + 1) * 512],
                    kt_sb[:, t * P : (t + 1) * P],
                    qt_sb[:, j * 512 : (j + 1) * 512],
                    start=True,
                    stop=True,
                )
            msk = msk_pool.tile([P, SQ], f32, tag="masked")
            nc.vector.scalar_tensor_tensor(
                msk[:],
                m_sb[:, t * SQ : (t + 1) * SQ],
                NEG,
                st[:],
                op0=mybir.AluOpType.mult,
                op1=mybir.AluOpType.add,
            )
            p = p_pool.tile([P, SQ], bf16, tag="p")
            nc.scalar.activation(p[:], msk[:], mybir.ActivationFunctionType.Exp)
            for j in range(2):
                nc.tensor.matmul(
                    rs[:, j * 512 : (j + 1) * 512],
                    ones_col[:],
                    p[:, j * 512 : (j + 1) * 512],
                    start=(t == 0),
                    stop=(t == KTILES - 1),
                )
                nc.tensor.matmul(
                    ot[:, j * 512 : (j + 1) * 512],
                    v_sb[:, t * DV : (t + 1) * DV],
                    p[:, j * 512 : (j + 1) * 512],
                    start=(t == 0),
                    stop=(t == KTILES - 1),
                )

        # --- normalize and store ---
        rcp = stat.tile([1, SQ], f32)
        nc.vector.reciprocal(rcp[:], rs[:])
        rep = st_ps.tile([P, SQ], f32, tag="st")  # 1/rowsum replicated to 128 rows
        for j in range(2):
            nc.tensor.matmul(
                rep[:, j * 512 : (j + 1) * 512],
                ones_row[:],
                rcp[:, j * 512 : (j + 1) * 512],
                start=True,
                stop=True,
            )
        rep_sb = stat.tile([P, SQ], f32)
        nc.scalar.copy(rep_sb[:], rep[:])
        ot_n = stat.tile([P, SQ], f32)
        nc.vector.tensor_tensor(ot_n[:], ot[:], rep_sb[:], op=mybir.AluOpType.mult)
        nc.sync.dma_start(out[:, :], ot_n[:])

    nc.compile()
    return nc


_NC_CACHE = None


def kernel(inputs, mask, Wq, Wk, Wv):
    global _NC_CACHE, LAST_RESULT
    inputs = np.asarray(inputs)
    mask = np.asarray(mask)
    bf = ml_dtypes.bfloat16
    scale = np.float32(1.0 / math.sqrt(DK))
    wq_h = (np.asarray(Wq) * scale).astype(bf)
    wk_h = np.asarray(Wk).astype(bf)
    wv_h = np.asarray(Wv).astype(bf)

    if _NC_CACHE is None:
        _NC_CACHE = build()
    nc = _NC_CACHE

    in_maps = []
    for core in range(8):
        b, h = divmod(core, 2)
        q0 = h * SQ
        idx = np.r_[q0:S, 0:q0]  # rotate so this core's queries come first
        xb = inputs[b]  # [S, E] f32
        xt_core = np.ascontiguousarray(xb[idx].T).astype(bf)  # [E, S]
        mt_core = np.ascontiguousarray(
            mask[b, q0 : q0 + SQ, :][:, idx].T
        ).astype(bf)  # [S, SQ]
        in_maps.append(
            {"xt": xt_core, "wq": wq_h, "wk": wk_h, "wv": wv_h, "mt": mt_core}
        )

    res = run_bass_kernel_spmd(nc, in_maps, list(range(8)), trace=TRACE)
    LAST_RESULT = res
    outp = np.empty((B, S, DV), np.float32)
    for core in range(8):
        b, h = divmod(core, 2)
        q0 = h * SQ
        outp[b, q0 : q0 + SQ, :] = np.asarray(res.results[core]["out"]).T
    return outp
